# revision 27
# baseline (speedup 1.0000x reference)
"""Trainium2 Bass kernel for nn_MixedAttention_16561393893612.

Computation (reference semantics, fp32 inputs):
  x [B=4, T=2048, D=1024]; first n_s=1984 tokens share QKV weights W_s
  [3D, D]; the last 64 tokens each have their own W_ns[t] [3D, D]; full
  softmax attention (H=16 heads, Dh=64) over all T; out-proj W_out [D, D].

Sharding: tensor-parallel over heads. Core c owns heads (2c, 2c+1):
  - QKV projection for its 128-row m-slice of W_s / W_ns (all tokens)
  - full attention for its 2 heads
  - out-proj partial with the matching 128-column slice of W_out;
    host sums the 8 partial outputs.

Layout strategy (all heavy matmuls contract over n=D on partitions):
  - host pre-transposes x (xT), the W_s/W_ns/W_out slices, so no
    on-chip transposes are needed;
  - Q^T/K^T are produced [m(128=2 heads), t] so scores come out as
    S^T [k, q] with softmax along free axis impossible -- instead we
    exp() without max-subtraction (inputs are unit-scale; scores*0.125
    are bounded ~|4|) and get the softmax denominator from an appended
    ones-column in V via the same PV matmul (row 64 of psO = sum_k P).
  - PV uses V in token-major layout [k, dh+1] as lhsT giving O^T [d, q],
    which feeds the out-proj directly (contraction over d).

Compute dtype bf16 (PE streams 1 elem/cycle; fp32 would be 4 passes),
fp32 PSUM accumulation everywhere, fp32 output partials.
"""

import os
import numpy as np
import ml_dtypes

import bass_rust
import concourse.bass as bass
import concourse.mybir as mybir
import concourse.tile as tile
from concourse.bass_utils import run_bass_kernel_spmd

FP32 = mybir.dt.float32
BF16 = mybir.dt.bfloat16
NPBF16 = ml_dtypes.bfloat16

B, T, D = 4, 2048, 1024
H, DH = 16, 64
NS = 64
N_S = T - NS  # 1984
NCORES = 8
HPC = H // NCORES     # heads per core = 2
M = HPC * DH          # 128: per-core m-slice width per q/k/v
NT = D // 128         # 8 n-tiles (contraction)
KT = T // 128         # 16 k-tiles
QTW = 512             # q tile width
QT = T // QTW         # 4 q tiles
SCALE = 1.0 / np.sqrt(DH).astype(np.float32)

# The walrus build on this image rejects instructions carrying more than
# one sync wait ("Too many sync wait commands").  Tile freely emits
# multi-wait instructions, so after tracing we hoist excess waits onto
# injected same-engine NoOps placed immediately before the instruction
# (each engine executes its block-order subsequence, so the waits still
# complete before the instruction issues).
_MAX_WAITS = 1


def _split_waits(nc, max_waits=_MAX_WAITS):
    ctr = 0
    for f in nc.m.functions:
        for blk in f.blocks:
            newlist = []
            for inst in blk.instructions:
                si = inst.sync_info
                waits = list(si.on_wait) if si else []
                if len(waits) > max_waits:
                    head, keep = waits[:-max_waits], waits[-max_waits:]
                    for i in range(0, len(head), max_waits):
                        chunk = head[i : i + max_waits]
                        nop = mybir.InstNoOp(name=f"W-split-{ctr}", ins=[], outs=[])
                        ctr += 1
                        nop.engine = inst.engine
                        nop.sync_info = mybir.SyncInfo(on_wait=chunk, on_update=[])
                        newlist.append(nop)
                    inst.sync_info = mybir.SyncInfo(
                        on_wait=keep, on_update=list(si.on_update)
                    )
                newlist.append(inst)
            blk.instructions[:] = newlist
    return ctr


def _build_program():
    nc = bass.Bass()
    xT_d = nc.dram_tensor("xT", [B, NT, 128, T], BF16, kind="ExternalInput")
    wq_d = nc.dram_tensor("wq", [128, NT, M], BF16, kind="ExternalInput")
    wk_d = nc.dram_tensor("wk", [128, NT, M], BF16, kind="ExternalInput")
    wv_d = nc.dram_tensor("wv", [128, NT, M], BF16, kind="ExternalInput")
    wo_d = nc.dram_tensor("wo", [DH, HPC, D], BF16, kind="ExternalInput")
    wns_d = nc.dram_tensor("wns", [NS, 128, NT, 3 * M], BF16, kind="ExternalInput")
    xns_d = nc.dram_tensor("xns", [128, NS, NT, B], BF16, kind="ExternalInput")
    y_d = nc.dram_tensor("y", [B, T, D], FP32, kind="ExternalOutput")
    ns_scr = nc.dram_tensor("ns_scratch", [B, NS, 3 * M], BF16)
    l_scr = nc.dram_tensor("l_scratch", [B, QT, HPC, QTW], FP32)
    lr_scr = nc.dram_tensor("lr_scratch", [B, QT, HPC, QTW], FP32)
    opre_scr = nc.dram_tensor("opre_scratch", [B, QT - 1, HPC, DH + 1, QTW], FP32)

    from contextlib import ExitStack

    with tile.TileContext(nc) as tc, ExitStack() as ctx:
        sing = ctx.enter_context(tc.tile_pool(name="sing", bufs=1))
        xpool = ctx.enter_context(tc.tile_pool(name="xpool", bufs=9))
        wnspool = ctx.enter_context(tc.tile_pool(name="wnspool", bufs=5))
        ptpool = ctx.enter_context(tc.tile_pool(name="ptpool", bufs=4))
        evac = ctx.enter_context(tc.tile_pool(name="evac", bufs=3))
        otp = ctx.enter_context(tc.tile_pool(name="otp", bufs=2))
        small = ctx.enter_context(tc.tile_pool(name="small", bufs=4))
        otup = ctx.enter_context(tc.tile_pool(name="otup", bufs=4))
        recbp = ctx.enter_context(tc.tile_pool(name="recbp", bufs=2))
        ps_big = ctx.enter_context(tc.tile_pool(name="ps_big", bufs=2, space="PSUM"))
        ps_pj = ctx.enter_context(tc.tile_pool(name="ps_pj", bufs=2, space="PSUM"))
        ps_o = ctx.enter_context(tc.tile_pool(name="ps_o", bufs=2, space="PSUM"))

        # ---- constants / persistent tensors ----
        wq_sb = sing.tile([128, NT, M], BF16)
        wk_sb = sing.tile([128, NT, M], BF16)
        wv_sb = sing.tile([128, NT, M], BF16)
        wo_sb = sing.tile([DH, HPC, D], BF16)
        nc.sync.dma_start(wq_sb, wq_d[:])
        nc.sync.dma_start(wk_sb, wk_d[:])
        nc.sync.dma_start(wv_sb, wv_d[:])
        nc.sync.dma_start(wo_sb, wo_d[:])

        QT_sb = sing.tile([M, B, T], BF16)          # [m(q rows), b, t]
        KT_sb = sing.tile([M, B, T], BF16)          # [m(k rows), b, t]
        V_sb = sing.tile([128, B, KT, HPC, DH + 1], BF16)  # token-major V
        nc.gpsimd.memset(V_sb[:, :, :, :, DH : DH + 1], 1.0)

        xns_sb = sing.tile([128, NS, NT, B], BF16)
        nc.sync.dma_start(xns_sb, xns_d[:])
        from concourse.masks import make_identity

        ident = sing.tile([128, 128], BF16)
        make_identity(nc, ident)

        # ---- ns projection: out[b, m] per ns-token, lhsT = x cols ----
        # results staged to DRAM so they can be re-loaded transposed.
        # Emitted in groups interleaved with the per-batch projections so
        # the 50MB wns stream overlaps projection compute instead of
        # blocking the PE queue at the head of the kernel.
        def ns_group(tp_lo, tp_hi):
            for tp in range(tp_lo, tp_hi):
                wt = wnspool.tile(
                    [128, NT, 3 * M], BF16, tag="wns", name=f"wns_{tp}"
                )
                nc.sync.dma_start(wt, wns_d[tp])
                psn = ps_pj.tile([B, 3 * M], FP32, tag="pj", name=f"psn_{tp}")
                for nt in range(NT):
                    nc.tensor.matmul(
                        psn,
                        lhsT=xns_sb[:, tp, nt, :],
                        rhs=wt[:, nt, :],
                        start=(nt == 0),
                        stop=(nt == NT - 1),
                    )
                nst = small.tile([B, 3 * M], BF16, tag="nst", name=f"nst_{tp}")
                nc.vector.tensor_copy(nst, psn)
                nc.sync.dma_start(ns_scr[:, tp, :], nst)

        # one (kt-pair, head) step of attention: two S matmuls into a
        # 2-bank psum, one batched exp, two PV accumulations
        def attn_ktp(b, qt, psO, ktp, kt_first, kt_last):
            kts = (2 * ktp, 2 * ktp + 1)
            psS = [
                ps_big.tile(
                    [128, 2 * QTW], FP32, tag="mm", name=f"psS_{b}_{qt}_{ktp}_{h}"
                )
                for h in range(HPC)
            ]
            for j, kt in enumerate(kts):
                for h in range(HPC):
                    nc.tensor.matmul(
                        psS[h][:, j * QTW : (j + 1) * QTW],
                        lhsT=KT_sb[
                            h * DH : (h + 1) * DH, b, kt * 128 : (kt + 1) * 128
                        ],
                        rhs=QT_sb[
                            h * DH : (h + 1) * DH, b, qt * QTW : (qt + 1) * QTW
                        ],
                        start=True,
                        stop=True,
                    )
            for h in range(HPC):
                pt = ptpool.tile([128, 2 * QTW], BF16, tag="pt")
                nc.scalar.activation(
                    pt, psS[h], mybir.ActivationFunctionType.Exp, scale=float(SCALE)
                )
                for j, kt in enumerate(kts):
                    nc.tensor.matmul(
                        psO[h],
                        lhsT=V_sb[:, b, kt, h, :],
                        rhs=pt[:, j * QTW : (j + 1) * QTW],
                        start=(kt == kt_first),
                        stop=(kt == kt_last),
                    )

        GRP = NS // B
        for b in range(B):
            # ---- shared QKV projection for batch b ----
            proj_scope = nc.named_scope(f"proj_b{b}")
            proj_scope.__enter__()
            xts = []
            for nt in range(NT):
                xt = xpool.tile([128, T], BF16, tag="xt")
                nc.sync.dma_start(xt, xT_d[b, nt])
                xts.append(xt)
            for w_sb, out_sb in ((wq_sb, QT_sb), (wk_sb, KT_sb)):
                for qt in range(QT):
                    ps = ps_pj.tile([M, QTW], FP32, tag="pj")
                    for nt in range(NT):
                        nc.tensor.matmul(
                            ps,
                            lhsT=w_sb[:, nt, :],
                            rhs=xts[nt][:, qt * QTW : (qt + 1) * QTW],
                            start=(nt == 0),
                            stop=(nt == NT - 1),
                        )
                    nc.vector.tensor_copy(
                        out_sb[:, b, qt * QTW : (qt + 1) * QTW], ps
                    )
            for tch in range(KT):
                ps = ps_pj.tile([128, M], FP32, tag="pj")
                for nt in range(NT):
                    nc.tensor.matmul(
                        ps,
                        lhsT=xts[nt][:, tch * 128 : (tch + 1) * 128],
                        rhs=wv_sb[:, nt, :],
                        start=(nt == 0),
                        stop=(nt == NT - 1),
                    )
                nc.vector.tensor_copy(
                    V_sb[:, b, tch, :, 0:DH],
                    ps.rearrange("p (h d) -> p h d", h=HPC),
                )

            # pre-patch attention: q-tiles 0..2 x k-tiles 0..13 touch no
            # ns tokens, so they can run during the DMA-bound head phase;
            # partial [65, 512] accumulators spill to DRAM
            for qt in range(QT - 1):
                psO = [
                    ps_o.tile(
                        [DH + 1, QTW], FP32, tag="psO", name=f"psOp_{b}_{qt}_{h}"
                    )
                    for h in range(HPC)
                ]
                for ktp in range(KT // 2 - 1):
                    attn_ktp(b, qt, psO, ktp, 0, KT - 3)
                for h in range(HPC):
                    otu = otup.tile(
                        [DH + 1, QTW], FP32, tag=f"otu{h}", name=f"otup_{b}_{qt}_{h}"
                    )
                    nc.vector.tensor_copy(otu, psO[h])
                    nc.gpsimd.dma_start(opre_scr[b, qt, h], otu)
            # interleave a quarter of the ns-token projections per batch
            # (keeps the wns DMA stream flowing under projection compute)
            ns_group(b * GRP, (b + 1) * GRP)
            proj_scope.__exit__(None, None, None)

        # ---- patch ns tokens (last 64) from the staged ns results ----
        # Q/K need a [t', m] -> [m, t'] transpose: PE-transpose beats a
        # 2-byte-granularity DMA gather by ~40x here
        for b in range(B):
            for j, out_sb in ((0, QT_sb), (1, KT_sb)):
                nsp = small.tile([NS, M], BF16, tag="nsp", name=f"nsp_{b}_{j}")
                nc.sync.dma_start(nsp, ns_scr[b, :, j * M : (j + 1) * M])
                pst = ps_pj.tile([M, NS], BF16, tag="pj", name=f"pst_{b}_{j}")
                nc.tensor.transpose(pst, nsp, ident[0:NS, 0:NS])
                nc.vector.tensor_copy(out_sb[:, b, N_S:T], pst)
            for h in range(HPC):
                nc.sync.dma_start(
                    V_sb[DH : 2 * DH, b, KT - 1, h, 0:DH],
                    ns_scr[b, :, 2 * M + h * DH : 2 * M + (h + 1) * DH],
                )

        for b in range(B):
            # ---- attention for batch b (2 heads) ----
            attn_scope = nc.named_scope(f"attn_b{b}")
            attn_scope.__enter__()
            OT = [otp.tile([DH, T], BF16, tag=f"ot{h}", name=f"ot{h}_{b}") for h in range(HPC)]
            otus = [[None] * QT for _ in range(HPC)]
            for qt in range(QT):
                psO = [ps_o.tile([DH + 1, QTW], FP32, tag="psO", name=f"psO_{b}_{qt}_{h}") for h in range(HPC)]
                if qt < QT - 1:
                    # only the last kt-pair remained (k-tile 15 holds the
                    # patched ns tokens); merge with the spilled partials
                    attn_ktp(b, qt, psO, KT // 2 - 1, KT - 2, KT - 1)
                else:
                    for ktp in range(KT // 2):
                        attn_ktp(b, qt, psO, ktp, 0, KT - 1)
                for h in range(HPC):
                    # evacuate the whole [65, 512] accumulator at once so the
                    # PSUM bank frees immediately; row 64 is the softmax sum l
                    otu = otup.tile(
                        [DH + 1, QTW], FP32, tag=f"otu{h}", name=f"otu_{b}_{qt}_{h}"
                    )
                    nc.vector.tensor_copy(otu, psO[h])
                    if qt < QT - 1:
                        # DMA-accumulate the spilled pre-patch partial onto
                        # the freshly evacuated post-patch partial
                        nc.gpsimd.dma_start(
                            otu, opre_scr[b, qt, h], accum_op=mybir.AluOpType.add
                        )
                    # spill the softmax sum row; the reciprocal+broadcast is
                    # batched per (b, h) below
                    nc.sync.dma_start(
                        l_scr[b, qt, h, :].rearrange("(o q) -> o q", o=1),
                        otu[DH : DH + 1, :],
                    )
                    otus[h][qt] = otu
            # ---- batched softmax normalization (one chain per head) ----
            for h in range(HPC):
                lpar = small.tile([128, QT, QTW // 128], FP32, tag="lpar")
                nc.sync.dma_start(
                    lpar,
                    l_scr[b, :, h, :].rearrange("q (p f) -> p q f", p=128),
                )
                nc.vector.reciprocal(lpar, lpar)
                nc.sync.dma_start(
                    lr_scr[b, :, h, :].rearrange("q (p f) -> p q f", p=128), lpar
                )
                recb = recbp.tile([DH, QT, QTW], FP32, tag="recb")
                src = lr_scr[b, :, h, :]
                nc.sync.dma_start(
                    recb,
                    bass.AP(
                        tensor=src.tensor,
                        offset=src.offset,
                        ap=[[0, DH]] + [list(a) for a in src.ap],
                    ),
                )
                for qt in range(QT):
                    nc.vector.tensor_mul(
                        OT[h][:, qt * QTW : (qt + 1) * QTW],
                        otus[h][qt][0:DH, :],
                        recb[:, qt, :],
                    )
            attn_scope.__exit__(None, None, None)
            # ---- out-projection partial for batch b ----
            oproj_scope = nc.named_scope(f"oproj_b{b}")
            oproj_scope.__enter__()
            for tch in range(KT):
                yt = evac.tile([128, D], FP32, tag="y")
                for e in range(D // QTW):
                    psY = ps_pj.tile([128, QTW], FP32, tag="pj")
                    for h in range(HPC):
                        nc.tensor.matmul(
                            psY,
                            lhsT=OT[h][:, tch * 128 : (tch + 1) * 128],
                            rhs=wo_sb[:, h, e * QTW : (e + 1) * QTW],
                            start=(h == 0),
                            stop=(h == HPC - 1),
                        )
                    # split evacuation across DVE and ACT to balance engines
                    if e % 2 == 0:
                        nc.vector.tensor_copy(yt[:, e * QTW : (e + 1) * QTW], psY)
                    else:
                        nc.scalar.activation(
                            yt[:, e * QTW : (e + 1) * QTW],
                            psY,
                            mybir.ActivationFunctionType.Copy,
                        )
                # one paired 512KB write per token chunk, on the idle
                # GPSIMD-issued queue
                nc.gpsimd.dma_start(
                    y_d[b, tch * 128 : (tch + 1) * 128, :], yt
                )
            oproj_scope.__exit__(None, None, None)

    _split_waits(nc)
    return nc


_NC_CACHE = None
LAST_RESULTS = None


def _prep_inputs(x, W_s, W_ns, W_out):
    """Slice/transpose/cast the full inputs into per-core input maps."""
    x = np.asarray(x, dtype=np.float32)
    W_s = np.asarray(W_s, dtype=np.float32)
    W_ns = np.asarray(W_ns, dtype=np.float32)
    W_out = np.asarray(W_out, dtype=np.float32)

    xb = x.astype(NPBF16)
    # xT[b, nt, p, t] = x[b, t, nt*128+p]
    xT = np.ascontiguousarray(
        xb.transpose(0, 2, 1).reshape(B, NT, 128, T)
    )
    # xns[p, t', nt, b] = x[b, n_s+t', nt*128+p]
    xns = np.ascontiguousarray(
        xb[:, N_S:, :].transpose(2, 1, 0).reshape(NT, 128, NS, B).transpose(1, 2, 0, 3)
    )
    wnsb = W_ns.astype(NPBF16)
    wsb = W_s.astype(NPBF16)
    wob = W_out.astype(NPBF16)

    in_maps = []
    for c in range(NCORES):
        r0 = c * M
        sel = np.r_[r0 : r0 + M, D + r0 : D + r0 + M, 2 * D + r0 : 2 * D + r0 + M]

        def wslice(rows):
            # [128 rows m, 1024 n] -> [128 p(n), NT, m]
            w = wsb[rows, :]  # [M, D]
            return np.ascontiguousarray(
                w.T.reshape(NT, 128, M).transpose(1, 0, 2)
            )

        wq = wslice(slice(r0, r0 + M))
        wk = wslice(slice(D + r0, D + r0 + M))
        wv = wslice(slice(2 * D + r0, 2 * D + r0 + M))
        # wo[p, h, e] = W_out[e, c*128 + h*64 + p]
        wo = np.ascontiguousarray(
            wob[:, c * M : (c + 1) * M].T.reshape(HPC, DH, D).transpose(1, 0, 2)
        )
        # wns[t', p, nt, m] = W_ns[t', sel[m], n=nt*128+p]
        wns = np.ascontiguousarray(
            wnsb[:, sel, :].transpose(0, 2, 1).reshape(NS, NT, 128, 3 * M).transpose(0, 2, 1, 3)
        )
        in_maps.append(
            {"xT": xT, "wq": wq, "wk": wk, "wv": wv, "wo": wo, "wns": wns, "xns": xns}
        )
    return in_maps


def kernel(x, n_s, W_s, W_ns, W_out):
    global _NC_CACHE, LAST_RESULTS
    assert int(n_s) == N_S, f"kernel compiled for n_s={N_S}, got {int(n_s)}"
    in_maps = _prep_inputs(x, W_s, W_ns, W_out)
    if _NC_CACHE is None:
        _NC_CACHE = _build_program()
    nc = _NC_CACHE
    trace = os.environ.get("BASS_TRACE", "") not in ("", "0")
    kwargs = {}
    if trace:
        stitch = os.environ.get("BASS_STITCH", "") not in ("", "0")
        kwargs = dict(
            trace=True, trace_cores=list(range(NCORES)), stitch_traces=stitch
        )
    res = run_bass_kernel_spmd(nc, in_maps, core_ids=list(range(NCORES)), **kwargs)
    LAST_RESULTS = res
    out = np.zeros((B, T, D), dtype=np.float32)
    for c in range(NCORES):
        out += res.results[c]["y"]
    return out
